# revision 2
# baseline (speedup 1.0000x reference)
"""MoE expert-gating kernel for 8 Trainium2 NeuronCores.

Problem (nn_ExpertGating): router MLP (H->H relu, H->E) + softmax + top-2
gating + weighted combine of per-expert outputs.

Sharding: data-parallel over the B*S=8192 tokens -> 1024 tokens per core.
Each core runs the full router for its tokens and combines its slice of all
8 experts' outputs.  No collectives needed; host concatenates the slices.

Per-core pipeline (T=1024 tokens, H=1024, E=8):
  1. x arrives host-pre-transposed (xT [H, T]) -- no PE transposes needed.
  2. hT = relu(W1.T @ xT + b1) in plain fp32 matmuls.  TRN2's fp32 matmul
     (LOW_HIGH) splits the stationary operand into two FP22 planes and
     streams the moving operand at full precision: 2 cycles/row, exact to
     fp32 -- cheaper than 3 full-rate fp16/bf16 passes (3 cycles/row) and
     far more accurate.  Measured on the v1 trace: fp32 N=512 matmul =
     450ns ~= 2*512/2.4GHz; back-to-back fp16 pairs = 214ns with LDW fully
     hidden, so LDW stays hidden here too (fp32 LDW ~140ns < N=256 213ns).
  3. logitsT[e, t] = W2.T @ hT + b2 (fp32, W2 stationary).
  4. transpose logit chunks back to [t, E] via PE (8x8 identity), softmax,
     top-2 via max8 + max_index -> top-2 gate values + expert ids.
  5. indirect-DMA gather of each token's 2 selected expert rows (8 MB
     instead of 32 MB dense), combine out[t] = g0*row0 + g1*row1.

Segments of 2 chunks (256 tokens) pipeline stage2 -> stage3 -> gather so
gathers start ~1/4 into stage2.  Segment 0 runs its k-loop outermost so
matmuls consume W1 k-blocks as the DMA delivers them (no serial DMA head).
fp32-accurate logits are required: min top-2/3 margin on this data is
~5e-6; fp32 keeps logit error ~1e-7.
"""

import numpy as np

B, S, H, E = 4, 2048, 1024, 8
N_CORES = 8
T = (B * S) // N_CORES  # tokens per core
P = 128  # partitions
TCH = T // P  # token chunks per core (8)
KT = H // P  # contraction tiles (8)
HAL = 512  # psum pad width (full bank)
SEGS = [(0, 2), (2, 4), (4, 6), (6, 8)]

_compiled_nc = None


def _build():
    import concourse.bacc as bacc
    import concourse.bass as bass
    import concourse.tile as tile
    from concourse import mybir

    f32 = mybir.dt.float32
    u32 = mybir.dt.uint32
    nc = bacc.Bacc("TRN2", target_bir_lowering=False, debug=False,
                   num_devices=N_CORES)

    xT = nc.dram_tensor("xT", [H, T], f32, kind="ExternalInput").ap()
    eo = nc.dram_tensor("eo", [E * T, H], f32, kind="ExternalInput").ap()
    w1 = nc.dram_tensor("w1", [H, H], f32, kind="ExternalInput").ap()
    b1 = nc.dram_tensor("b1", [H], f32, kind="ExternalInput").ap()
    w2 = nc.dram_tensor("w2", [H, E], f32, kind="ExternalInput").ap()
    b2 = nc.dram_tensor("b2", [E], f32, kind="ExternalInput").ap()
    identd = nc.dram_tensor("ident", [E, E], f32, kind="ExternalInput").ap()
    iotad = nc.dram_tensor("iota", [P, 1], u32, kind="ExternalInput").ap()
    out = nc.dram_tensor("out", [T, H], f32, kind="ExternalOutput").ap()

    with tile.TileContext(nc) as tc:
        with (
            tc.tile_pool(name="singles", bufs=1) as singles,
            tc.tile_pool(name="eopool", bufs=4) as eopool,
            tc.tile_pool(name="accpool", bufs=3) as accpool,
            tc.tile_pool(name="smalls", bufs=8) as smalls,
            tc.tile_pool(name="ltpool", bufs=2) as ltpool,
            tc.tile_pool(name="psum", bufs=8, space="PSUM") as psum,
        ):
            # small constants first on the SP ring; the Pool/SWDGE ring is
            # reserved for the 16 gathers
            ident = singles.tile([E, E], f32)
            nc.sync.dma_start(out=ident[:], in_=identd)
            iota_u = singles.tile([P, 1], u32)
            nc.sync.dma_start(out=iota_u[:], in_=iotad)
            b1_sb = singles.tile([P, KT], f32)  # b1_sb[p,m] = b1[m*128+p]
            nc.sync.dma_start(out=b1_sb[:], in_=b1.rearrange("(m p) -> p m", p=P))
            b2_sb = singles.tile([E, 1], f32)
            nc.sync.dma_start(out=b2_sb[:], in_=b2[:, None])
            w2_sb = singles.tile([P, KT, E], f32)  # w2_sb[p,j,e] = W2[j*128+p, e]
            nc.sync.dma_start(out=w2_sb[:], in_=w2.rearrange("(j p) e -> p j e", p=P))

            # xT seg0 + W1, interleaved by k so seg0's k-outer matmul loop
            # can start as soon as the first k-block lands
            xT_sb = singles.tile([P, KT, T], f32)  # xT_sb[p,k,t] = x[t, k*128+p]
            w1_sb = singles.tile([P, KT, H], f32)  # w1_sb[p,k,m] = W1[k*128+p, m]
            S0 = SEGS[0][1] * P  # seg0 token width
            for k in range(KT):
                nc.sync.dma_start(
                    out=xT_sb[:, k, 0:S0],
                    in_=xT[k * P:(k + 1) * P, 0:S0])
                nc.sync.dma_start(
                    out=w1_sb[:, k, :],
                    in_=w1[k * P:(k + 1) * P, :])
            # remaining xT segments
            for c0, c1 in SEGS[1:]:
                for k in range(KT):
                    nc.sync.dma_start(
                        out=xT_sb[:, k, c0 * P:c1 * P],
                        in_=xT[k * P:(k + 1) * P, c0 * P:c1 * P])

            hT = singles.tile([P, KT, T], f32)  # hT[p,m,t] = relu(x@W1+b1)[t, m*128+p]

            for si, (c0, c1) in enumerate(SEGS):
                sl = slice(c0 * P, c1 * P)
                W = (c1 - c0) * P
                # ---- stage 2: hT = relu(W1.T @ xT + b1), fp32 ----
                if si == 0:
                    # k-outer: consume W1/xT k-blocks as the DMA delivers them
                    ps_m = [psum.tile([P, W], f32, tag="ps", name=f"ps{m}",
                                      padded_shape=[P, HAL]) for m in range(KT)]
                    for k in range(KT):
                        for m in range(KT):
                            nc.tensor.matmul(
                                ps_m[m][:],
                                lhsT=w1_sb[:, k, m * P:(m + 1) * P],
                                rhs=xT_sb[:, k, sl],
                                start=(k == 0), stop=(k == KT - 1),
                            )
                    for m in range(KT):
                        nc.scalar.activation(
                            out=hT[:, m, sl], in_=ps_m[m][:],
                            func=mybir.ActivationFunctionType.Relu,
                            bias=b1_sb[:, m:m + 1], scale=1.0,
                        )
                else:
                    for m in range(KT):
                        ps = psum.tile([P, W], f32, tag="ps", name="ps",
                                       padded_shape=[P, HAL])
                        for k in range(KT):
                            nc.tensor.matmul(
                                ps[:],
                                lhsT=w1_sb[:, k, m * P:(m + 1) * P],
                                rhs=xT_sb[:, k, sl],
                                start=(k == 0), stop=(k == KT - 1),
                            )
                        nc.scalar.activation(
                            out=hT[:, m, sl], in_=ps[:],
                            func=mybir.ActivationFunctionType.Relu,
                            bias=b1_sb[:, m:m + 1], scale=1.0,
                        )

                # ---- stage 3: logitsT[e, seg] = W2.T @ hT (+ b2) ----
                ps3 = psum.tile([E, W], f32, tag="ps", name="ps3",
                                padded_shape=[E, HAL])
                for j in range(KT):
                    nc.tensor.matmul(
                        ps3[:], lhsT=w2_sb[:, j, :], rhs=hT[:, j, sl],
                        start=(j == 0), stop=(j == KT - 1),
                    )
                lT = ltpool.tile([E, W], f32, tag="lT", name="lT",
                                 padded_shape=[E, HAL])
                nc.scalar.activation(out=lT[:], in_=ps3[:],
                                     func=mybir.ActivationFunctionType.Identity,
                                     bias=b2_sb[:, 0:1], scale=1.0)

                # ---- stage 4+5 per 128-token chunk: softmax, top-2,
                # indirect gather of the 2 selected expert rows, combine ----
                for tch in range(c0, c1):
                    a = tch - c0
                    pl = psum.tile([P, E], f32, tag="ps", name="pl",
                                   padded_shape=[P, HAL])
                    nc.tensor.transpose(pl[:], lT[:, a * P:(a + 1) * P],
                                        ident[:])
                    negmax = smalls.tile([P, 1], f32, tag="negmax", name="negmax")
                    nc.vector.reduce_max(negmax[:], pl[:],
                                         axis=mybir.AxisListType.X, negate=True)
                    exps = smalls.tile([P, E], f32, tag="exps", name="exps")
                    nc.scalar.activation(exps[:], pl[:],
                                         func=mybir.ActivationFunctionType.Exp,
                                         bias=negmax[:], scale=1.0)
                    ssum = smalls.tile([P, 1], f32, tag="ssum", name="ssum")
                    nc.vector.reduce_sum(ssum[:], exps[:],
                                         axis=mybir.AxisListType.X)
                    rs = smalls.tile([P, 1], f32, tag="rs", name="rs")
                    nc.vector.reciprocal(rs[:], ssum[:])
                    # top-2 of exps == top-2 of probs; gate = exp * (1/sum)
                    mx8 = smalls.tile([P, 8], f32, tag="mx8", name="mx8")
                    nc.vector.max(mx8[:], exps[:])
                    idx8 = smalls.tile([P, 8], u32, tag="idx8", name="idx8")
                    nc.vector.max_index(idx8[:], mx8[:], exps[:])
                    # flat eo row = expert*T + (tch*128 + partition)
                    base = smalls.tile([P, 1], u32, tag="base", name="base")
                    nc.vector.tensor_scalar_add(base[:], iota_u[:], tch * P)
                    rows = smalls.tile([P, 2], u32, tag="rows", name="rows")
                    for s in range(2):
                        nc.vector.tensor_scalar(
                            rows[:, s:s + 1], idx8[:, s:s + 1],
                            scalar1=T, scalar2=None, op0=mybir.AluOpType.mult)
                        nc.vector.tensor_tensor(
                            out=rows[:, s:s + 1], in0=rows[:, s:s + 1],
                            in1=base[:], op=mybir.AluOpType.add)
                    eo_g = eopool.tile([P, 2, H], f32, tag="eog", name="eog")
                    for s in range(2):
                        nc.gpsimd.indirect_dma_start(
                            out=eo_g[:, s, :], out_offset=None, in_=eo,
                            in_offset=bass.IndirectOffsetOnAxis(
                                ap=rows[:, s:s + 1], axis=0))
                    g0 = smalls.tile([P, 1], f32, tag="g0", name="g0")
                    nc.vector.tensor_mul(g0[:], mx8[:, 0:1], rs[:])
                    g1 = smalls.tile([P, 1], f32, tag="g1", name="g1")
                    nc.vector.tensor_mul(g1[:], mx8[:, 1:2], rs[:])
                    acc = accpool.tile([P, H], f32, tag="acc", name="acc")
                    nc.scalar.activation(acc[:], eo_g[:, 0, :],
                                         func=mybir.ActivationFunctionType.Copy,
                                         scale=g0[:])
                    nc.vector.scalar_tensor_tensor(
                        out=acc[:], in0=eo_g[:, 1, :], scalar=g1[:], in1=acc[:],
                        op0=mybir.AluOpType.mult, op1=mybir.AluOpType.add)
                    nc.sync.dma_start(out=out[tch * P:(tch + 1) * P, :],
                                      in_=acc[:])

    nc.compile()
    return nc


def _get_nc():
    global _compiled_nc
    if _compiled_nc is None:
        _compiled_nc = _build()
    return _compiled_nc


def make_in_maps(hidden_states, expert_outputs, W1, b1, W2, b2):
    hs = np.ascontiguousarray(np.asarray(hidden_states, dtype=np.float32)).reshape(B * S, H)
    eo = np.ascontiguousarray(np.asarray(expert_outputs, dtype=np.float32)).reshape(E, B * S, H)
    w1 = np.ascontiguousarray(np.asarray(W1, dtype=np.float32))
    b1v = np.ascontiguousarray(np.asarray(b1, dtype=np.float32))
    w2 = np.ascontiguousarray(np.asarray(W2, dtype=np.float32))
    b2v = np.ascontiguousarray(np.asarray(b2, dtype=np.float32))
    identv = np.eye(E, dtype=np.float32)
    iotav = np.arange(P, dtype=np.uint32).reshape(P, 1)
    in_maps = []
    for c in range(N_CORES):
        sl = slice(c * T, (c + 1) * T)
        in_maps.append({
            "xT": np.ascontiguousarray(hs[sl].T),
            "eo": np.ascontiguousarray(eo[:, sl, :]).reshape(E * T, H),
            "w1": w1, "b1": b1v, "w2": w2, "b2": b2v,
            "ident": identv, "iota": iotav,
        })
    return in_maps


def kernel(hidden_states, expert_outputs, W1, b1, W2, b2, k=2):
    from concourse.bass_utils import run_bass_kernel_spmd

    in_maps = make_in_maps(hidden_states, expert_outputs, W1, b1, W2, b2)
    nc = _get_nc()
    res = run_bass_kernel_spmd(nc, in_maps, core_ids=list(range(N_CORES)))
    full = np.concatenate([res.results[c]["out"] for c in range(N_CORES)], axis=0)
    return full.reshape(B, S, H)


# revision 5
# speedup vs baseline: 1.0861x; 1.0861x over previous
"""MoE expert-gating kernel for 8 Trainium2 NeuronCores.

Problem (nn_ExpertGating): router MLP (H->H relu, H->E) + softmax + top-2
gating + weighted combine of per-expert outputs.

Sharding: data-parallel over the B*S=8192 tokens -> 1024 tokens per core.
Each core runs the full router for its tokens and combines its slice of all
8 experts' outputs.  No collectives needed; host concatenates the slices.

Per-core pipeline (T=1024 tokens, H=1024, E=8):
  1. x arrives host-pre-transposed AND pre-split into fp16 hi + bf16 lo
     halves (xThi/xTlo [H, T]) -- no PE transposes, no on-device splits.
  2. hT = relu(W1.T @ xT + b1) via 3 fp16/bf16 matmul passes (hi*hi;
     hi*lo + lo*hi), 1 cycle/row each.  This is the PE floor: one side of
     every matmul is limited to FP22 (~12-13 bit) operands, and both x and
     W1 need ~19 bits for the logits to rank top-2 correctly (min top-2/3
     margin on this data is ~5e-6; fp16x3 keeps logit error ~1e-6).
     fp32 matmul is 4 cycles/row (2 HW instructions) and f32r truncates
     both sides to ~12 bits (probed on HW), so neither beats 3x fp16.
  3. logitsT[e, t] += W2.T @ hT accumulated per m-block right after each
     relu (fp32 stationary: W2 also needs >13 bits), so segment logits are
     ready immediately after the segment's last stage-2 matmul.
  4. transpose logit chunks back to [t, E] via PE (8x8 identity), softmax,
     top-2 via max8 + max_index -> top-2 gate values + expert ids.
  5. indirect-DMA gather of each token's 2 selected expert rows (8 MB
     instead of 32 MB dense), combine out[t] = g0*row0 + g1*row1.

All segments are 256 tokens wide: fp16 LDW (~97ns) stays hidden under the
N=256 matmul (107ns), and gathers for segment s overlap stage 2 of s+1.
Segment 0 runs k-outermost so matmuls consume W1/xT k-blocks as the DMA
delivers them instead of waiting for all of W1.
"""

import numpy as np

B, S, H, E = 4, 2048, 1024, 8
N_CORES = 8
T = (B * S) // N_CORES  # tokens per core
P = 128  # partitions
TCH = T // P  # token chunks per core (8)
KT = H // P  # contraction tiles (8)
HAL = 512  # psum pad width (full bank)
SEGS = [(0, 2), (2, 4), (4, 6), (6, 8)]

_compiled_nc = None


def _build():
    import concourse.bacc as bacc
    import concourse.bass as bass
    import concourse.tile as tile
    from concourse import mybir

    f32 = mybir.dt.float32
    f16 = mybir.dt.float16
    bf16 = mybir.dt.bfloat16
    u32 = mybir.dt.uint32
    nc = bacc.Bacc("TRN2", target_bir_lowering=False, debug=False,
                   num_devices=N_CORES)

    xh = nc.dram_tensor("xh", [H, T], f16, kind="ExternalInput").ap()
    xl = nc.dram_tensor("xl", [H, T], bf16, kind="ExternalInput").ap()
    eo = nc.dram_tensor("eo", [E * T, H], f32, kind="ExternalInput").ap()
    w1h = nc.dram_tensor("w1h", [H, H], f16, kind="ExternalInput").ap()
    w1l = nc.dram_tensor("w1l", [H, H], bf16, kind="ExternalInput").ap()
    b1 = nc.dram_tensor("b1", [H], f32, kind="ExternalInput").ap()
    w2 = nc.dram_tensor("w2", [H, E], f32, kind="ExternalInput").ap()
    b2 = nc.dram_tensor("b2", [E], f32, kind="ExternalInput").ap()
    identd = nc.dram_tensor("ident", [E, E], f32, kind="ExternalInput").ap()
    iotad = nc.dram_tensor("iota", [P, 1], u32, kind="ExternalInput").ap()
    out = nc.dram_tensor("out", [T, H], f32, kind="ExternalOutput").ap()

    with tile.TileContext(nc) as tc:
        with (
            tc.tile_pool(name="singles", bufs=1) as singles,
            tc.tile_pool(name="eopool", bufs=4) as eopool,
            tc.tile_pool(name="accpool", bufs=3) as accpool,
            tc.tile_pool(name="smalls", bufs=8) as smalls,
            tc.tile_pool(name="ltpool", bufs=2) as ltpool,
            tc.tile_pool(name="psum", bufs=8, space="PSUM") as psum,
        ):
            # small constants first on the SP ring; the Pool/SWDGE ring is
            # reserved for the 16 gathers
            ident = singles.tile([E, E], f32)
            nc.sync.dma_start(out=ident[:], in_=identd)
            iota_u = singles.tile([P, 1], u32)
            nc.sync.dma_start(out=iota_u[:], in_=iotad)
            b1_sb = singles.tile([P, KT], f32)  # b1_sb[p,m] = b1[m*128+p]
            nc.sync.dma_start(out=b1_sb[:], in_=b1.rearrange("(m p) -> p m", p=P))
            b2_sb = singles.tile([E, 1], f32)
            nc.sync.dma_start(out=b2_sb[:], in_=b2[:, None])
            w2_sb = singles.tile([P, KT, E], f32)  # w2_sb[p,j,e] = W2[j*128+p, e]
            nc.sync.dma_start(out=w2_sb[:], in_=w2.rearrange("(j p) e -> p j e", p=P))

            # seg0's xT + W1, interleaved by k so seg0's k-outer matmul loop
            # starts as soon as the first k-block lands
            xh_sb = singles.tile([P, KT, T], f16)   # xh_sb[p,k,t] = fp16 hi of x[t, k*128+p]
            xl_sb = singles.tile([P, KT, T], bf16)  # bf16 lo (unscaled)
            w1h_sb = singles.tile([P, KT, H], f16)  # w1h_sb[p,k,m] = fp16 hi of W1[k*128+p, m]
            w1l_sb = singles.tile([P, KT, H], bf16)
            S0 = SEGS[0][1] * P  # seg0 token width
            for k in range(KT):
                ksl = slice(k * P, (k + 1) * P)
                nc.sync.dma_start(out=xh_sb[:, k, 0:S0], in_=xh[ksl, 0:S0])
                nc.sync.dma_start(out=xl_sb[:, k, 0:S0], in_=xl[ksl, 0:S0])
                nc.sync.dma_start(out=w1h_sb[:, k, :], in_=w1h[ksl, :])
                nc.sync.dma_start(out=w1l_sb[:, k, :], in_=w1l[ksl, :])
            for c0, c1 in SEGS[1:]:
                for k in range(KT):
                    ksl = slice(k * P, (k + 1) * P)
                    nc.sync.dma_start(out=xh_sb[:, k, c0 * P:c1 * P],
                                      in_=xh[ksl, c0 * P:c1 * P])
                    nc.sync.dma_start(out=xl_sb[:, k, c0 * P:c1 * P],
                                      in_=xl[ksl, c0 * P:c1 * P])

            hT = singles.tile([P, KT, T], f32)  # hT[p,m,t] = relu(x@W1+b1)[t, m*128+p]

            def mm3(ps, m, k, sl, start, stop):
                msl = slice(m * P, (m + 1) * P)
                nc.tensor.matmul(ps[:], lhsT=w1h_sb[:, k, msl],
                                 rhs=xh_sb[:, k, sl], start=start, stop=False)
                nc.tensor.matmul(ps[:], lhsT=w1l_sb[:, k, msl],
                                 rhs=xh_sb[:, k, sl], start=False, stop=False)
                nc.tensor.matmul(ps[:], lhsT=w1h_sb[:, k, msl],
                                 rhs=xl_sb[:, k, sl], start=False, stop=stop)

            for si, (c0, c1) in enumerate(SEGS):
                sl = slice(c0 * P, c1 * P)
                W = (c1 - c0) * P
                # ---- stage 2: hT = relu(W1.T @ xT + b1), fp16 x3 ----
                # ---- stage 3 (interleaved): logitsT += W2[j].T @ hT[j] ----
                if si == 0:
                    # k-outer: consume W1/xT k-blocks as the DMA delivers them.
                    # ps3 must be allocated AFTER the 8 stage-2 accumulators:
                    # the pool has exactly 8 slots, and a 9th live tile ahead
                    # of them would deadlock the slot ring on hardware.
                    ps_m = [psum.tile([P, W], f32, tag="ps", name=f"ps{m}",
                                      padded_shape=[P, HAL]) for m in range(KT)]
                    for k in range(KT):
                        for m in range(KT):
                            mm3(ps_m[m], m, k, sl, k == 0, k == KT - 1)
                    ps3 = psum.tile([E, W], f32, tag="ps", name="ps3",
                                    padded_shape=[E, HAL])
                    for m in range(KT):
                        nc.scalar.activation(
                            out=hT[:, m, sl], in_=ps_m[m][:],
                            func=mybir.ActivationFunctionType.Relu,
                            bias=b1_sb[:, m:m + 1], scale=1.0,
                        )
                        nc.tensor.matmul(
                            ps3[:], lhsT=w2_sb[:, m, :], rhs=hT[:, m, sl],
                            start=(m == 0), stop=(m == KT - 1),
                        )
                else:
                    ps3 = psum.tile([E, W], f32, tag="ps", name="ps3",
                                    padded_shape=[E, HAL])
                    for m in range(KT):
                        ps = psum.tile([P, W], f32, tag="ps", name="ps",
                                       padded_shape=[P, HAL])
                        for k in range(KT):
                            mm3(ps, m, k, sl, k == 0, k == KT - 1)
                        nc.scalar.activation(
                            out=hT[:, m, sl], in_=ps[:],
                            func=mybir.ActivationFunctionType.Relu,
                            bias=b1_sb[:, m:m + 1], scale=1.0,
                        )
                        nc.tensor.matmul(
                            ps3[:], lhsT=w2_sb[:, m, :], rhs=hT[:, m, sl],
                            start=(m == 0), stop=(m == KT - 1),
                        )

                lT = ltpool.tile([E, W], f32, tag="lT", name="lT",
                                 padded_shape=[E, HAL])
                nc.scalar.activation(out=lT[:], in_=ps3[:],
                                     func=mybir.ActivationFunctionType.Identity,
                                     bias=b2_sb[:, 0:1], scale=1.0)

                # ---- stage 4+5 per 128-token chunk: softmax, top-2,
                # indirect gather of the 2 selected expert rows, combine ----
                for tch in range(c0, c1):
                    a = tch - c0
                    pl = psum.tile([P, E], f32, tag="ps", name="pl",
                                   padded_shape=[P, HAL])
                    nc.tensor.transpose(pl[:], lT[:, a * P:(a + 1) * P],
                                        ident[:])
                    negmax = smalls.tile([P, 1], f32, tag="negmax", name="negmax")
                    nc.vector.reduce_max(negmax[:], pl[:],
                                         axis=mybir.AxisListType.X, negate=True)
                    exps = smalls.tile([P, E], f32, tag="exps", name="exps")
                    nc.scalar.activation(exps[:], pl[:],
                                         func=mybir.ActivationFunctionType.Exp,
                                         bias=negmax[:], scale=1.0)
                    ssum = smalls.tile([P, 1], f32, tag="ssum", name="ssum")
                    nc.vector.reduce_sum(ssum[:], exps[:],
                                         axis=mybir.AxisListType.X)
                    rs = smalls.tile([P, 1], f32, tag="rs", name="rs")
                    nc.vector.reciprocal(rs[:], ssum[:])
                    # top-2 of exps == top-2 of probs; gate = exp * (1/sum)
                    mx8 = smalls.tile([P, 8], f32, tag="mx8", name="mx8")
                    nc.vector.max(mx8[:], exps[:])
                    idx8 = smalls.tile([P, 8], u32, tag="idx8", name="idx8")
                    nc.vector.max_index(idx8[:], mx8[:], exps[:])
                    # flat eo row = expert*T + (tch*128 + partition)
                    base = smalls.tile([P, 1], u32, tag="base", name="base")
                    nc.vector.tensor_scalar_add(base[:], iota_u[:], tch * P)
                    rows = smalls.tile([P, 2], u32, tag="rows", name="rows")
                    for s in range(2):
                        nc.vector.tensor_scalar(
                            rows[:, s:s + 1], idx8[:, s:s + 1],
                            scalar1=T, scalar2=None, op0=mybir.AluOpType.mult)
                        nc.vector.tensor_tensor(
                            out=rows[:, s:s + 1], in0=rows[:, s:s + 1],
                            in1=base[:], op=mybir.AluOpType.add)
                    eo_g = eopool.tile([P, 2, H], f32, tag="eog", name="eog")
                    for s in range(2):
                        nc.gpsimd.indirect_dma_start(
                            out=eo_g[:, s, :], out_offset=None, in_=eo,
                            in_offset=bass.IndirectOffsetOnAxis(
                                ap=rows[:, s:s + 1], axis=0))
                    g0 = smalls.tile([P, 1], f32, tag="g0", name="g0")
                    nc.vector.tensor_mul(g0[:], mx8[:, 0:1], rs[:])
                    g1 = smalls.tile([P, 1], f32, tag="g1", name="g1")
                    nc.vector.tensor_mul(g1[:], mx8[:, 1:2], rs[:])
                    acc = accpool.tile([P, H], f32, tag="acc", name="acc")
                    nc.scalar.activation(acc[:], eo_g[:, 0, :],
                                         func=mybir.ActivationFunctionType.Copy,
                                         scale=g0[:])
                    nc.vector.scalar_tensor_tensor(
                        out=acc[:], in0=eo_g[:, 1, :], scalar=g1[:], in1=acc[:],
                        op0=mybir.AluOpType.mult, op1=mybir.AluOpType.add)
                    nc.sync.dma_start(out=out[tch * P:(tch + 1) * P, :],
                                      in_=acc[:])

    nc.compile()
    return nc


def _get_nc():
    global _compiled_nc
    if _compiled_nc is None:
        _compiled_nc = _build()
    return _compiled_nc


def _split_hi_lo(a):
    """fp16 hi + bf16 lo split of an fp32 array (lo unscaled; bf16's
    exponent range covers it)."""
    import ml_dtypes
    a = np.asarray(a, dtype=np.float32)
    hi = a.astype(np.float16)
    lo = (a.astype(np.float64) - hi.astype(np.float64)).astype(ml_dtypes.bfloat16)
    return np.ascontiguousarray(hi), np.ascontiguousarray(lo)


def make_in_maps(hidden_states, expert_outputs, W1, b1, W2, b2):
    hs = np.ascontiguousarray(np.asarray(hidden_states, dtype=np.float32)).reshape(B * S, H)
    eo = np.ascontiguousarray(np.asarray(expert_outputs, dtype=np.float32)).reshape(E, B * S, H)
    w1hi, w1lo = _split_hi_lo(W1)
    b1v = np.ascontiguousarray(np.asarray(b1, dtype=np.float32))
    w2 = np.ascontiguousarray(np.asarray(W2, dtype=np.float32))
    b2v = np.ascontiguousarray(np.asarray(b2, dtype=np.float32))
    identv = np.eye(E, dtype=np.float32)
    iotav = np.arange(P, dtype=np.uint32).reshape(P, 1)
    in_maps = []
    for c in range(N_CORES):
        sl = slice(c * T, (c + 1) * T)
        xhi, xlo = _split_hi_lo(hs[sl].T)
        in_maps.append({
            "xh": xhi, "xl": xlo,
            "eo": np.ascontiguousarray(eo[:, sl, :]).reshape(E * T, H),
            "w1h": w1hi, "w1l": w1lo, "b1": b1v, "w2": w2, "b2": b2v,
            "ident": identv, "iota": iotav,
        })
    return in_maps


def kernel(hidden_states, expert_outputs, W1, b1, W2, b2, k=2):
    from concourse.bass_utils import run_bass_kernel_spmd

    in_maps = make_in_maps(hidden_states, expert_outputs, W1, b1, W2, b2)
    nc = _get_nc()
    res = run_bass_kernel_spmd(nc, in_maps, core_ids=list(range(N_CORES)))
    full = np.concatenate([res.results[c]["out"] for c in range(N_CORES)], axis=0)
    return full.reshape(B, S, H)
